# revision 11
# baseline (speedup 1.0000x reference)
"""Bass/Trainium2 kernel for nn_Attn_13846974562399.

Computes, for the reference module:
    proj   = enc @ W^T + bias          # [S, B, H]
    scores = einsum('bh,sbh->bs', hidden[0], proj)
    attn   = softmax(scores, axis=1)   # -> [B, 1, S]

Algebraic restructure used here:
    scores[b, s] = q[b] . enc[s, b] + (hidden[0,b] . bias)
    with q = hidden[0] @ W.
The per-b constant (hidden . bias) is invariant under softmax over s, so it
is dropped.  q ([B, H], ~128 KB) is computed on the host in float64; the
memory-bound work -- streaming the 268 MB encoder tensor and the batched
dot-products -- runs on 8 NeuronCores, data-parallel over the batch dim
(4 batches per core).

Per-core device program (DMA-bound, ~358 GB/s/core HBM roofline ~94 us):
  - 4x 512 KB q-replica loads (issued first so the first STT is gated only
    by the first encoder tile).
  - 15 tile loads of 2 MB each (fully contiguous; host pre-permutes the
    shard to [t, p, b, h] with s = p*16 + t), plus the last tile split
    into 4 per-b 512 KB loads so the final dot-products pipeline.
  - 64 fused DVE scalar_tensor_tensor ops (out=(enc*1)*q, accum_out=
    sum_h) -> scores[p, b, t].  (TENSOR_TENSOR_REDUCE crashes this
    runtime's NX ucode; scalar_tensor_tensor is the same fused ALU path.)
  - Softmax over the 2048 scores per b, per-b chains interleaved with the
    last tile's STTs: per-partition max (DVE) -> cross-partition max
    (GPSIMD all-reduce) -> exp with per-partition bias and fused free-dim
    sum (ACT) -> cross-partition sum (GPSIMD) -> reciprocal + scale (DVE)
    -> per-b 8 KB DMA out.
"""

import numpy as np

import concourse.bacc as bacc
import concourse.bass as bass
import concourse.mybir as mybir
import concourse.tile as tile
from concourse.bass_isa import ReduceOp
from concourse.bass_utils import run_bass_kernel_spmd

S, B, H = 2048, 32, 1024
NCORES = 8
BL = B // NCORES          # 4 local batches per core
P = 128                   # SBUF partitions
NT = S // P               # 16 s-tiles; s = p*NT + t
F32 = mybir.dt.float32

ENC_BUFS = 6              # in-flight 2 MB encoder tiles

# Populated by the most recent kernel() call (for test harnesses).
LAST_RESULTS = None
TRACE = False

_NC = None


def _build_bass():
    nc = bacc.Bacc()
    enc = nc.dram_tensor("enc", [NT, P, BL, H], F32, kind="ExternalInput")
    q = nc.dram_tensor("q", [1, BL * H], F32, kind="ExternalInput")
    out = nc.dram_tensor("attn", [P, BL, NT], F32, kind="ExternalOutput")

    mult = mybir.AluOpType.mult

    with tile.TileContext(nc) as tc:
        with (
            tc.tile_pool(name="encp", bufs=ENC_BUFS) as enc_pool,
            tc.tile_pool(name="firstp", bufs=BL) as first_pool,
            tc.tile_pool(name="lastp", bufs=BL) as last_pool,
            tc.tile_pool(name="small", bufs=1) as small,
            tc.tile_pool(name="psum", bufs=1, space="PSUM") as psum_pool,
        ):
            # On-device q broadcast: 16 KB DMA (first in the FIFO ring), then
            # PE replicates it across partitions (ones[1,128].T @ q_chunk ->
            # PSUM), and ACT copies per-b slices to SBUF.  Keeps the 2 MB
            # replica out of the HBM stream and off gpsimd (whose custom-op
            # library load is ~9 us into the kernel).
            q0 = small.tile([1, BL * H], F32)
            nc.sync.dma_start(out=q0, in_=q.ap())
            ones = small.tile([1, P], F32)
            nc.vector.memset(ones, 1.0)
            qps = psum_pool.tile([P, BL * H], F32)
            MMN = 512  # fp32 moving-operand max
            for k in range(BL * H // MMN):
                nc.tensor.matmul(
                    qps[:, k * MMN : (k + 1) * MMN],
                    ones[:],
                    q0[:, k * MMN : (k + 1) * MMN],
                    start=True,
                    stop=True,
                )
            qb = small.tile([P, BL, H], F32)
            for b in range(BL):
                nc.scalar.copy(out=qb[:, b, :], in_=qps[:, b * H : (b + 1) * H])

            scores = small.tile([P, BL, NT], F32)
            dummy = small.tile([P, 1], F32)
            enc_ap = enc.ap()

            def stt(et_b, b, t):
                # out = (enc * 1.0) * q; accum_out = sum over h.
                nc.vector.scalar_tensor_tensor(
                    out=dummy.broadcast_to((P, H)),
                    in0=et_b,
                    scalar=1.0,
                    in1=qb[:, b, :],
                    op0=mult,
                    op1=mult,
                    accum_out=scores[:, b, t : t + 1],
                )

            # First tile split per-b (512 KB chunks) so the first STT starts
            # as soon as ~1 MB of the FIFO ring has landed.
            for b in range(BL):
                etf = first_pool.tile([P, H], F32)
                nc.sync.dma_start(out=etf, in_=enc_ap[0, :, b, :])
                stt(etf[:], b, 0)

            for t in range(1, NT - 1):
                et = enc_pool.tile([P, BL, H], F32)
                nc.sync.dma_start(out=et, in_=enc_ap[t])
                for b in range(BL):
                    stt(et[:, b, :], b, t)

            # Softmax working tiles.
            m = small.tile([P, BL], F32)
            negm = small.tile([P, BL], F32)
            e = small.tile([P, BL, NT], F32)
            ssum = small.tile([P, BL], F32)
            rz = small.tile([P, BL], F32)
            attn_sb = small.tile([P, BL, NT], F32)

            # Last tile: per-b 512 KB loads; kick off the per-b score max
            # (DVE) + cross-partition max (GPSIMD) behind each final STT.
            t = NT - 1
            for b in range(BL):
                etl = last_pool.tile([P, H], F32)
                nc.sync.dma_start(out=etl, in_=enc_ap[t, :, b, :])
                stt(etl[:], b, t)
                nc.vector.tensor_reduce(
                    out=m[:, b : b + 1],
                    in_=scores[:, b, :],
                    axis=mybir.AxisListType.X,
                    op=mybir.AluOpType.max,
                )
                nc.gpsimd.partition_all_reduce(
                    m[:, b : b + 1], m[:, b : b + 1], P, ReduceOp.max
                )

            for b in range(BL):
                nc.vector.tensor_scalar_mul(
                    out=negm[:, b : b + 1], in0=m[:, b : b + 1], scalar1=-1.0
                )
                nc.scalar.activation(
                    out=e[:, b, :],
                    in_=scores[:, b, :],
                    func=mybir.ActivationFunctionType.Exp,
                    bias=negm[:, b : b + 1],
                    scale=1.0,
                    accum_out=ssum[:, b : b + 1],
                )
                nc.gpsimd.partition_all_reduce(
                    ssum[:, b : b + 1], ssum[:, b : b + 1], P, ReduceOp.add
                )
            for b in range(BL):
                nc.vector.reciprocal(rz[:, b : b + 1], ssum[:, b : b + 1])
                nc.vector.tensor_scalar_mul(
                    out=attn_sb[:, b, :], in0=e[:, b, :], scalar1=rz[:, b : b + 1]
                )
                nc.sync.dma_start(out=out.ap()[:, b, :], in_=attn_sb[:, b, :])

    nc.compile()
    return nc


def kernel(hidden, encoder_outputs, W, b):
    global _NC, LAST_RESULTS
    hidden = np.asarray(hidden, dtype=np.float32)
    enc = np.asarray(encoder_outputs, dtype=np.float32)
    W = np.asarray(W, dtype=np.float32)

    # q = hidden[0] @ W  (fp64 accumulate on host; tiny vs the 268 MB stream).
    # The bias term contributes a per-b constant to the scores, which softmax
    # cancels, so `b` is unused.
    q_full = (hidden[0].astype(np.float64) @ W.astype(np.float64)).astype(np.float32)

    in_maps = []
    for c in range(NCORES):
        enc_c = enc[:, BL * c : BL * (c + 1), :]            # [S, BL, H]
        # [t, p, b, h] with s = p*NT + t
        enc_r = np.ascontiguousarray(
            enc_c.reshape(P, NT, BL, H).transpose(1, 0, 2, 3)
        )
        q_c = np.ascontiguousarray(
            q_full[BL * c : BL * (c + 1)].reshape(1, BL * H)
        )
        in_maps.append({"enc": enc_r, "q": q_c})

    if _NC is None:
        _NC = _build_bass()

    LAST_RESULTS = run_bass_kernel_spmd(
        _NC, in_maps, core_ids=list(range(NCORES)), trace=TRACE
    )

    out = np.empty((B, 1, S), dtype=np.float32)
    for c in range(NCORES):
        a = LAST_RESULTS.results[c]["attn"]                 # [P, BL, NT]
        out[BL * c : BL * (c + 1), 0, :] = a.transpose(1, 0, 2).reshape(BL, S)
    return out


# revision 12
# speedup vs baseline: 1.0911x; 1.0911x over previous
"""Bass/Trainium2 kernel for nn_Attn_13846974562399.

Computes, for the reference module:
    proj   = enc @ W^T + bias          # [S, B, H]
    scores = einsum('bh,sbh->bs', hidden[0], proj)
    attn   = softmax(scores, axis=1)   # -> [B, 1, S]

Algebraic restructure used here:
    scores[b, s] = q[b] . enc[s, b] + (hidden[0,b] . bias)
    with q = hidden[0] @ W.
The per-b constant (hidden . bias) is invariant under softmax over s, so it
is dropped.  q ([B, H], ~128 KB) is computed on the host in float64; the
memory-bound work -- streaming the 268 MB encoder tensor and the batched
dot-products -- runs on 8 NeuronCores, data-parallel over the batch dim
(4 batches per core).

Per-core device program (DMA-bound; ~358 GB/s/core HBM roofline ~94 us):
  - The host pre-permutes the shard to [t, b, p, h] with s = p*16 + t, so
    every (t, b) unit is one fully contiguous 512 KB read.  All loads go
    through the sync-engine HWDGE ring, which drains FIFO: 4 q-replica
    chunks interleaved with the first tile's chunks, then the remaining 60
    encoder chunks.  First dot-product starts ~3 us after streaming begins.
  - 64 fused DVE scalar_tensor_tensor ops (out=(enc*1)*q, accum_out=
    sum_h) -> scores[p, b, t].  (TENSOR_TENSOR_REDUCE crashes this
    runtime's NX ucode; scalar_tensor_tensor is the same fused ALU path.)
  - Softmax over the 2048 scores per b, per-b chains interleaved with the
    last tile's STTs: per-partition max (DVE) -> cross-partition max
    (GPSIMD all-reduce) -> exp with per-partition bias and fused free-dim
    sum (ACT) -> cross-partition sum (GPSIMD) -> reciprocal + scale (DVE)
    -> per-b 8 KB DMA out.
"""

import numpy as np

import concourse.bacc as bacc
import concourse.bass as bass
import concourse.mybir as mybir
import concourse.tile as tile
from concourse.bass_isa import ReduceOp
from concourse.bass_utils import run_bass_kernel_spmd

S, B, H = 2048, 32, 1024
NCORES = 8
BL = B // NCORES          # 4 local batches per core
P = 128                   # SBUF partitions
NT = S // P               # 16 s-tiles; s = p*NT + t
F32 = mybir.dt.float32

ENC_BUFS = 12             # in-flight 512 KB encoder chunks

# Populated by the most recent kernel() call (for test harnesses).
LAST_RESULTS = None
TRACE = False

_NC = None


def _build_bass():
    nc = bacc.Bacc()
    enc = nc.dram_tensor("enc", [NT, BL, P, H], F32, kind="ExternalInput")
    qrep = nc.dram_tensor("qrep", [BL, P, H], F32, kind="ExternalInput")
    out = nc.dram_tensor("attn", [P, BL, NT], F32, kind="ExternalOutput")

    mult = mybir.AluOpType.mult

    with tile.TileContext(nc) as tc:
        with (
            tc.tile_pool(name="encp", bufs=ENC_BUFS) as enc_pool,
            tc.tile_pool(name="small", bufs=1) as small,
        ):
            qb = small.tile([P, BL, H], F32)
            scores = small.tile([P, BL, NT], F32)
            dummy = small.tile([P, 1], F32)
            m = small.tile([P, BL], F32)
            negm = small.tile([P, BL], F32)
            e = small.tile([P, BL, NT], F32)
            ssum = small.tile([P, BL], F32)
            rz = small.tile([P, BL], F32)
            attn_sb = small.tile([P, BL, NT], F32)

            enc_ap = enc.ap()
            qrep_ap = qrep.ap()

            def stt(et, b, t):
                # out = (enc * 1.0) * q; accum_out = sum over h.
                nc.vector.scalar_tensor_tensor(
                    out=dummy.broadcast_to((P, H)),
                    in0=et[:],
                    scalar=1.0,
                    in1=qb[:, b, :],
                    op0=mult,
                    op1=mult,
                    accum_out=scores[:, b, t : t + 1],
                )

            for t in range(NT):
                for b in range(BL):
                    if t == 0:
                        # Interleave the q-replica chunk just ahead of the
                        # matching first-tile chunk on the FIFO ring.
                        nc.sync.dma_start(out=qb[:, b, :], in_=qrep_ap[b])
                    et = enc_pool.tile([P, H], F32)
                    nc.sync.dma_start(out=et, in_=enc_ap[t, b])
                    stt(et, b, t)
                    if t == NT - 1:
                        # Kick off this b's softmax stats right behind its
                        # final dot-product.
                        nc.vector.tensor_reduce(
                            out=m[:, b : b + 1],
                            in_=scores[:, b, :],
                            axis=mybir.AxisListType.X,
                            op=mybir.AluOpType.max,
                        )
                        nc.gpsimd.partition_all_reduce(
                            m[:, b : b + 1], m[:, b : b + 1], P, ReduceOp.max
                        )

            for b in range(BL):
                nc.vector.tensor_scalar_mul(
                    out=negm[:, b : b + 1], in0=m[:, b : b + 1], scalar1=-1.0
                )
                nc.scalar.activation(
                    out=e[:, b, :],
                    in_=scores[:, b, :],
                    func=mybir.ActivationFunctionType.Exp,
                    bias=negm[:, b : b + 1],
                    scale=1.0,
                    accum_out=ssum[:, b : b + 1],
                )
                nc.gpsimd.partition_all_reduce(
                    ssum[:, b : b + 1], ssum[:, b : b + 1], P, ReduceOp.add
                )
            for b in range(BL):
                nc.vector.reciprocal(rz[:, b : b + 1], ssum[:, b : b + 1])
                nc.vector.tensor_scalar_mul(
                    out=attn_sb[:, b, :], in0=e[:, b, :], scalar1=rz[:, b : b + 1]
                )
                nc.sync.dma_start(out=out.ap()[:, b, :], in_=attn_sb[:, b, :])

    nc.compile()
    return nc


def kernel(hidden, encoder_outputs, W, b):
    global _NC, LAST_RESULTS
    hidden = np.asarray(hidden, dtype=np.float32)
    enc = np.asarray(encoder_outputs, dtype=np.float32)
    W = np.asarray(W, dtype=np.float32)

    # q = hidden[0] @ W  (fp64 accumulate on host; tiny vs the 268 MB stream).
    # The bias term contributes a per-b constant to the scores, which softmax
    # cancels, so `b` is unused.
    q_full = (hidden[0].astype(np.float64) @ W.astype(np.float64)).astype(np.float32)

    in_maps = []
    for c in range(NCORES):
        enc_c = enc[:, BL * c : BL * (c + 1), :]            # [S, BL, H]
        # [t, b, p, h] with s = p*NT + t: every (t, b) is contiguous 512 KB.
        enc_r = np.ascontiguousarray(
            enc_c.reshape(P, NT, BL, H).transpose(1, 2, 0, 3)
        )
        q_c = q_full[BL * c : BL * (c + 1)]                 # [BL, H]
        q_rep = np.ascontiguousarray(
            np.broadcast_to(q_c[:, None, :], (BL, P, H))
        )
        in_maps.append({"enc": enc_r, "qrep": q_rep})

    if _NC is None:
        _NC = _build_bass()

    LAST_RESULTS = run_bass_kernel_spmd(
        _NC, in_maps, core_ids=list(range(NCORES)), trace=TRACE
    )

    out = np.empty((B, 1, S), dtype=np.float32)
    for c in range(NCORES):
        a = LAST_RESULTS.results[c]["attn"]                 # [P, BL, NT]
        out[BL * c : BL * (c + 1), 0, :] = a.transpose(1, 0, 2).reshape(BL, S)
    return out
